# revision 31
# baseline (speedup 1.0000x reference)
"""DCNv2 (modulated deformable conv 3x3) for Trainium2, 8 NeuronCores.

Sharding: pure data-parallel over batch B=8 -> core b computes batch b.

Per-core algorithm (batch b, C=Cout=128, H=W=64, P=H*W=4096):
  1. PE (fp32): offset/mask conv as 9 accumulated matmuls over a zero-padded
     channel-major x ([128, 66*66] SBUF), output [41, P] channel-major
     (channels: 0:9 y-offsets, 9:18 x-offsets, 32:41 mask - 32-aligned for
     the engines' base-partition restriction).  ACT applies bias (+ sigmoid
     for mask rows) during PSUM evacuation.
  2. PE transposes [41,128] chunks -> p-major planes [128(p), 41, 32(pb)].
  3. DVE: bilinear coefficient planes.  floor() via the fp32 round trick
     (x - 0.5 + 1.5*2^23) - 1.5*2^23 (ties resolve either way; bilinear
     interpolation is continuous so both splits give identical samples).
     Per kernel-point k one gather index  idx = ysel*64 + xsel  with
     ysel = clip(floor(py), 0, 62), xsel = clip(floor(px), 0, 62), and four
     per-corner coefficients  C[yl][xl] = mask * ylane_yl * xlane_xl  where
     the lane coefficients remap the fetched span (ysel..+1) x (xsel..+1)
     onto the true bilinear corners including border clip/zero semantics.
  4. GPSIMD dma_gather (transpose=True) over a host-packed bf16 table
     x2[p] = [x[p], x[p+1], x[p+64], x[p+65]] ([P, 512] in DRAM): each
     int16 index fetches 1KB = all four bilinear corners x 128 channels,
     landing transposed as four [c, p] planes.  One gather per k.
  5. PE (bf16): per (k, corner, p-block): Z^T[p,o] = G[c,p-blk].T @ W_k[c,o]
     (gathered block as the stationary operand) -> PSUM [128, 4x128].
  6. DVE accumulates acc[p, o] += coef_corner[p] * Z^T straight from PSUM
     via scalar_tensor_tensor (per-partition scalar = per-position coef).
  7. Output [P, 128] (p-major) DMAd out; host transposes to [Cout, H, W].
"""

import sys

sys.path.insert(0, "/opt/trn_rl_repo")

import numpy as np
import ml_dtypes

import concourse.bacc as bacc
import concourse.bass as bass
import concourse.mybir as mybir
import concourse.tile as tile
from concourse.ap import AP
from concourse.bass import ts
from concourse.bass_utils import run_bass_kernel_spmd
from concourse.library_config import mlp as mlp_lib
from concourse.masks import make_identity

F32 = mybir.dt.float32
BF16 = mybir.dt.bfloat16
I16 = mybir.dt.int16

B, C, H, W = 8, 128, 64, 64
COUT = 128
K2 = 9
P = H * W            # 4096
NPB = P // 128       # 32 p-blocks
HP = H + 2           # padded side
MAGIC = 12582912.0   # 1.5 * 2**23
AOP = mybir.AluOpType
AF = mybir.ActivationFunctionType

_CACHE = {}


def _build():
    nc = bacc.Bacc("TRN2", target_bir_lowering=False, num_swdge_queues=4)

    xpad_d = nc.dram_tensor("xpad", [128, HP * HP], F32, kind="ExternalInput")
    x2_d = nc.dram_tensor("x2rows", [P, 512], BF16, kind="ExternalInput")
    wmain_d = nc.dram_tensor("wmain", [K2, 128, COUT], BF16, kind="ExternalInput")
    woff_d = nc.dram_tensor("woff", [K2, 128, 41], F32, kind="ExternalInput")
    bias_d = nc.dram_tensor("bias41", [41, 1], F32, kind="ExternalInput")
    byk_d = nc.dram_tensor("byk", [128, K2, NPB], F32, kind="ExternalInput")
    bxk_d = nc.dram_tensor("bxk", [128, K2, NPB], F32, kind="ExternalInput")
    out_d = nc.dram_tensor("out", [P, COUT], F32, kind="ExternalOutput")

    with tile.TileContext(nc) as tc:
        with (
            tc.tile_pool(name="const", bufs=1) as cp,
            tc.tile_pool(name="coef", bufs=1) as cf,
            tc.tile_pool(name="gp", bufs=2) as gp,
        ):
            from contextlib import ExitStack
            _es0 = ExitStack()
            nc.gpsimd.load_library(mlp_lib)

            # ---- constant loads (SP-engine HWDGE queues, off gpsimd) ----
            xpad = cp.tile([128, HP * HP], F32)
            nc.sync.dma_start(xpad[:, 0 : 10 * HP], xpad_d[:, 0 : 10 * HP])
            nc.sync.dma_start(xpad[:, 10 * HP :], xpad_d[:, 10 * HP :])
            wm = cp.tile([128, K2, COUT], BF16)
            nc.sync.dma_start(wm[:], wmain_d[:].rearrange("k c o -> c k o"))
            wo = cp.tile([128, K2, 41], F32)
            nc.sync.dma_start(wo[:], woff_d[:].rearrange("k c j -> c k j"))
            bias = cp.tile([41, 1], F32)
            nc.sync.dma_start(bias[:], bias_d[:])
            byk = cp.tile([128, K2, NPB], F32)
            nc.sync.dma_start(byk[:], byk_d[:])
            bxk = cp.tile([128, K2, NPB], F32)
            nc.sync.dma_start(bxk[:], bxk_d[:])
            ident = cp.tile([64, 64], F32)
            make_identity(nc, ident[:])
            # PE warm-up: ~4us of dummy matmuls so the HAM un-throttles
            # before the offset conv begins.
            wup = _es0.enter_context(tc.tile_pool(name="wup", bufs=1, space="PSUM"))
            wps = wup.tile([64, 512], F32)
            for _ in range(8):
                nc.tensor.matmul(
                    wps[:, 0:64], ident[:], ident[:], start=True, stop=True
                )
            for i in range(16):
                nc.tensor.matmul(
                    wps[:], ident[:], xpad[0:64, 0:512], start=True, stop=True
                )

            # ---- offset/mask conv: [41, P] channel-major ----
            _es1 = ExitStack()
            psO_ctx = _es1.enter_context(tc.tile_pool(name="psO", bufs=2, space="PSUM"))
            psT_ctx = _es1.enter_context(tc.tile_pool(name="psT", bufs=2, space="PSUM"))
            offs_cm = cf.tile([41, P], F32)
            nc.gpsimd.memset(offs_cm[:], 0.0)
            xv = xpad[:].rearrange("c (h w) -> c h w", h=HP)
            for ch in range(8):
                po = psO_ctx.tile([41, 512], F32)
                r0 = ch * 8
                for k in range(K2):
                    ki, kj = k // 3, k % 3
                    rhs = xv[:, r0 + ki : r0 + ki + 8, kj : kj + W]
                    nc.tensor.matmul(
                        po[:], wo[:, k, :], rhs,
                        start=(k == 0), stop=(k == K2 - 1),
                    )
                sl = slice(ch * 512, (ch + 1) * 512)
                nc.scalar.activation(
                    offs_cm[0:18, sl], po[0:18, :], AF.Identity,
                    bias=bias[0:18, :], scale=1.0,
                )
                nc.scalar.activation(
                    offs_cm[32:41, sl], po[32:41, :], AF.Sigmoid,
                    bias=bias[32:41, :], scale=1.0,
                )

            # ---- transpose to p-major [128, 41, 32] ----
            offs_pm = cf.tile([128, 41, NPB], F32)
            for t in range(NPB):
                pt = psT_ctx.tile([128, 41], F32)
                nc.tensor.transpose(
                    pt[:], offs_cm[:, ts(t, 128)], ident[:41, :41]
                )
                nc.vector.tensor_copy(offs_pm[:, :, t], pt[:])

            offy = offs_pm[:, 0:9, :]
            offx = offs_pm[:, 9:18, :]
            mask = offs_pm[:, 32:41, :]

            # ---- coefficient planes (DVE, [128, 9, 32] each) ----
            SH = [128, K2, NPB]
            _tln = [0]

            def tl():
                _tln[0] += 1
                return cf.tile(SH, F32, name=f"cftmp{_tln[0]}")

            def TS(out, in0, s1, op0, s2=None, op1=None):
                kw = {"op1": op1} if op1 is not None else {}
                nc.vector.tensor_scalar(
                    out=out, in0=in0, scalar1=s1, scalar2=s2, op0=op0, **kw
                )

            def TT(out, a, b, op):
                nc.vector.tensor_tensor(out=out, in0=a, in1=b, op=op)

            # index chain first (gathers depend only on this)
            t0 = tl(); TS(t0[:], offy, -0.5, AOP.add, MAGIC, AOP.add)
            iy = tl(); TS(iy[:], t0[:], MAGIC, AOP.subtract)
            ys0 = tl(); TT(ys0[:], iy[:], byk[:], AOP.add)
            ysel = tl(); TS(ysel[:], ys0[:], 0.0, AOP.max, 62.0, AOP.min)
            t1 = tl(); TS(t1[:], offx, -0.5, AOP.add, MAGIC, AOP.add)
            ix = tl(); TS(ix[:], t1[:], MAGIC, AOP.subtract)
            xs0 = tl(); TT(xs0[:], ix[:], bxk[:], AOP.add)
            xst = tl(); TS(xst[:], xs0[:], 0.0, AOP.max, 62.0, AOP.min)
            ib = tl()
            nc.vector.scalar_tensor_tensor(
                out=ib[:], in0=ysel[:], scalar=64.0, in1=xst[:],
                op0=AOP.mult, op1=AOP.add,
            )
            idx16 = cf.tile([128, K2, NPB], I16)
            nc.vector.tensor_copy(idx16[:], ib[:])
            # wrap per-k across 3 HWDGE queues so gathers start asap
            idxw = cf.tile([128, K2, 256], I16)
            _dmaengs = (nc.sync, nc.sync)
            for k in range(K2):
                for g in range(8):
                    _dmaengs[g % 2].dma_start(
                        idxw[0:16, k, g:256:8],
                        idx16[16 * g : 16 * (g + 1), k, :],
                    )
                for qi, np2 in enumerate((16, 32, 64)):
                    _dmaengs[qi % 2].dma_start(
                        idxw[np2 : 2 * np2, k, :], idxw[0:np2, k, :]
                    )
            # remaining coefficient math
            fy = tl(); TT(fy[:], offy, iy[:], AOP.subtract)
            ys1 = tl(); TS(ys1[:], ys0[:], 1.0, AOP.add)
            yc0 = tl(); TS(yc0[:], ys0[:], 0.0, AOP.max, 63.0, AOP.min)
            yc1 = tl(); TS(yc1[:], ys1[:], 0.0, AOP.max, 63.0, AOP.min)
            vy0 = tl(); TT(vy0[:], yc0[:], ys0[:], AOP.is_equal)
            vy1 = tl(); TT(vy1[:], yc1[:], ys1[:], AOP.is_equal)
            gy = tl(); TS(gy[:], fy[:], -1.0, AOP.mult, 1.0, AOP.add)
            wy0 = tl(); TT(wy0[:], gy[:], vy0[:], AOP.mult)
            wy1 = tl(); TT(wy1[:], fy[:], vy1[:], AOP.mult)
            f0 = tl(); TT(f0[:], ysel[:], ys0[:], AOP.is_equal)
            fm = tl(); TS(fm[:], ys0[:], -1.0, AOP.is_equal)
            fp = tl(); TS(fp[:], ys0[:], 63.0, AOP.is_equal)
            ya = tl(); TT(ya[:], wy0[:], f0[:], AOP.mult)
            yb = tl(); TT(yb[:], wy1[:], fm[:], AOP.mult)
            ylane0 = tl(); TT(ylane0[:], ya[:], yb[:], AOP.add)
            yc_ = tl(); TT(yc_[:], wy1[:], f0[:], AOP.mult)
            yd = tl(); TT(yd[:], wy0[:], fp[:], AOP.mult)
            ylane1 = tl(); TT(ylane1[:], yc_[:], yd[:], AOP.add)
            myl0 = tl(); TT(myl0[:], ylane0[:], mask, AOP.mult)
            myl1 = tl(); TT(myl1[:], ylane1[:], mask, AOP.mult)
            # x side
            fx = tl(); TT(fx[:], offx, ix[:], AOP.subtract)
            xs1 = tl(); TS(xs1[:], xs0[:], 1.0, AOP.add)
            xc0 = tl(); TS(xc0[:], xs0[:], 0.0, AOP.max, 63.0, AOP.min)
            xc1 = tl(); TS(xc1[:], xs1[:], 0.0, AOP.max, 63.0, AOP.min)
            vx0 = tl(); TT(vx0[:], xc0[:], xs0[:], AOP.is_equal)
            vx1 = tl(); TT(vx1[:], xc1[:], xs1[:], AOP.is_equal)
            gx = tl(); TS(gx[:], fx[:], -1.0, AOP.mult, 1.0, AOP.add)
            wx0 = tl(); TT(wx0[:], gx[:], vx0[:], AOP.mult)
            wx1 = tl(); TT(wx1[:], fx[:], vx1[:], AOP.mult)
            e0 = tl(); TT(e0[:], xst[:], xs0[:], AOP.is_equal)
            em = tl(); TS(em[:], xs0[:], -1.0, AOP.is_equal)
            ep = tl(); TS(ep[:], xs0[:], 63.0, AOP.is_equal)
            l0a = tl(); TT(l0a[:], wx0[:], e0[:], AOP.mult)
            l0b = tl(); TT(l0b[:], wx1[:], em[:], AOP.mult)
            xlane0 = tl(); TT(xlane0[:], l0a[:], l0b[:], AOP.add)
            l1a = tl(); TT(l1a[:], wx1[:], e0[:], AOP.mult)
            l1b = tl(); TT(l1b[:], wx0[:], ep[:], AOP.mult)
            xlane1 = tl(); TT(xlane1[:], l1a[:], l1b[:], AOP.add)
            # final per-corner coefficients (gather quarter order:
            # (y0,x0), (y0,x1), (y1,x0), (y1,x1))
            C00 = tl(); TT(C00[:], myl0[:], xlane0[:], AOP.mult)
            C01 = tl(); TT(C01[:], myl0[:], xlane1[:], AOP.mult)
            C10 = tl(); TT(C10[:], myl1[:], xlane0[:], AOP.mult)
            C11 = tl(); TT(C11[:], myl1[:], xlane1[:], AOP.mult)

            _es1.close()
            _es0.close()
            _es2 = ExitStack()
            psZ = _es2.enter_context(tc.tile_pool(name="psZ", bufs=6, space="PSUM"))
            z2p = _es2.enter_context(tc.tile_pool(name="z2p", bufs=4))

            # ---- main loop ----
            acc = cf.tile([128, NPB, COUT], F32)
            nc.gpsimd.memset(acc[:], 0.0)

            src_ap = AP(
                tensor=x2_d[:].tensor, offset=0, ap=[[512, P], [1, 512]]
            )
            CPLANES = (C00, C01, C10, C11)
            NIDX_CHUNK = 512
            NCH = P // NIDX_CHUNK
            for k in range(K2):
                gt = gp.tile([128, NCH, 4, NIDX_CHUNK], BF16, tag="G")
                for c8 in range(NCH):
                    nc.gpsimd.dma_gather(
                        gt[:, c8, :, :],
                        src_ap,
                        idxw[:, k, c8 * 32 : (c8 + 1) * 32],
                        NIDX_CHUNK, NIDX_CHUNK,
                        elem_size=512, elem_step=512, transpose=True,
                        queue_num=0,
                    )
                for pb in range(NPB):
                    pz = psZ.tile([128, 512], F32)
                    c8, sub = pb // 4, pb % 4
                    for j in range(4):
                        nc.tensor.matmul(
                            pz[:, ts(j, 128)],
                            gt[:, c8, j, ts(sub, 128)],
                            wm[:, k, :],
                            start=True, stop=True,
                        )
                    # corners 0,1: ACT scaled-copies into PSUM (fast path);
                    # DVE: two fused STTs + two TTs, each with a single
                    # PSUM operand (walrus constraint)
                    z2 = z2p.tile([128, 2, 128], F32, tag="z2")
                    for j in range(2):
                        nc.scalar.activation(
                            z2[:, j, :], pz[:, ts(j, 128)], AF.Copy,
                            scale=CPLANES[j][:, k, pb : pb + 1],
                        )
                    u = z2p.tile([128, 2, 128], F32, tag="u")
                    for j in (2, 3):
                        nc.vector.scalar_tensor_tensor(
                            out=u[:, j - 2, :],
                            in0=pz[:, ts(j, 128)],
                            scalar=CPLANES[j][:, k, pb : pb + 1],
                            in1=z2[:, j - 2, :],
                            op0=AOP.mult, op1=AOP.add,
                        )
                    zt = z2p.tile([128, 128], F32, tag="zt")
                    nc.vector.tensor_tensor(
                        out=zt[:], in0=u[:, 0, :], in1=u[:, 1, :], op=AOP.add
                    )
                    nc.vector.tensor_tensor(
                        out=acc[:, pb, :], in0=zt[:], in1=acc[:, pb, :],
                        op=AOP.add,
                    )

            nc.sync.dma_start(
                out_d[:].rearrange("(pb part) o -> part pb o", part=128), acc[:]
            )
            _es2.close()

    nc.compile()
    return nc


def _host_prep(x, weight, offset_w, offset_b, mask_w, mask_b):
    x = np.asarray(x, np.float32)
    weight = np.asarray(weight, np.float32)
    offset_w = np.asarray(offset_w, np.float32)
    offset_b = np.asarray(offset_b, np.float32)
    mask_w = np.asarray(mask_w, np.float32)
    mask_b = np.asarray(mask_b, np.float32)

    wmain = np.ascontiguousarray(
        np.transpose(weight.reshape(COUT, C, K2), (2, 1, 0))
    ).astype(ml_dtypes.bfloat16)
    ow = offset_w.reshape(18, C, K2)
    w41 = np.zeros((41, C, K2), np.float32)
    w41[0:9] = ow[0::2]
    w41[9:18] = ow[1::2]
    w41[32:41] = mask_w.reshape(9, C, K2)
    woff = np.ascontiguousarray(np.transpose(w41, (2, 1, 0)))
    bias41 = np.zeros((41, 1), np.float32)
    bias41[0:9, 0] = offset_b[0::2]
    bias41[9:18, 0] = offset_b[1::2]
    bias41[32:41, 0] = mask_b

    ps = np.arange(P)
    ho = (ps // W).reshape(NPB, 128).T.astype(np.float32)
    wo_ = (ps % W).reshape(NPB, 128).T.astype(np.float32)
    byk = np.empty((128, K2, NPB), np.float32)
    bxk = np.empty((128, K2, NPB), np.float32)
    for k in range(K2):
        byk[:, k, :] = ho + (k // 3 - 1)
        bxk[:, k, :] = wo_ + (k % 3 - 1)

    shared = dict(wmain=wmain, woff=woff, bias41=bias41, byk=byk, bxk=bxk)

    in_maps = []
    for b in range(B):
        xpad = np.zeros((C, HP, HP), np.float32)
        xpad[:, 1 : H + 1, 1 : W + 1] = x[b]
        xr = np.zeros((P + 66, C), ml_dtypes.bfloat16)
        xr[:P] = x[b].transpose(1, 2, 0).reshape(P, C).astype(ml_dtypes.bfloat16)
        x2 = np.ascontiguousarray(
            np.concatenate(
                [xr[0:P], xr[1 : P + 1], xr[64 : P + 64], xr[65 : P + 65]],
                axis=1,
            )
        )
        in_maps.append(
            dict(xpad=xpad.reshape(C, HP * HP), x2rows=x2, **shared)
        )
    return in_maps


def kernel(x, weight, offset_w, offset_b, mask_w, mask_b):
    if "nc" not in _CACHE:
        _CACHE["nc"] = _build()
    nc = _CACHE["nc"]
    in_maps = _host_prep(x, weight, offset_w, offset_b, mask_w, mask_b)
    res = run_bass_kernel_spmd(nc, in_maps, list(range(B)))
    _CACHE["last_result"] = res
    out = np.empty((B, COUT, H, W), np.float32)
    for b in range(B):
        out[b] = res.results[b]["out"].T.reshape(COUT, H, W)
    return out


# revision 34
# speedup vs baseline: 1.0511x; 1.0511x over previous
"""DCNv2 (modulated deformable conv 3x3) for Trainium2, 8 NeuronCores.

Sharding: pure data-parallel over batch B=8 -> core b computes batch b.

Per-core algorithm (batch b, C=Cout=128, H=W=64, P=H*W=4096):
  1. PE (fp32): offset/mask conv as 9 accumulated matmuls over a zero-padded
     channel-major x ([128, 66*66] SBUF), output [41, P] channel-major
     (channels: 0:9 y-offsets, 9:18 x-offsets, 32:41 mask - 32-aligned for
     the engines' base-partition restriction).  ACT applies bias (+ sigmoid
     for mask rows) during PSUM evacuation.
  2. PE transposes [41,128] chunks -> p-major planes [128(p), 41, 32(pb)].
  3. DVE: bilinear coefficient planes.  floor() via the fp32 round trick
     (x - 0.5 + 1.5*2^23) - 1.5*2^23 (ties resolve either way; bilinear
     interpolation is continuous so both splits give identical samples).
     Per kernel-point k one gather index  idx = ysel*64 + xsel  with
     ysel = clip(floor(py), 0, 62), xsel = clip(floor(px), 0, 62), and four
     per-corner coefficients  C[yl][xl] = mask * ylane_yl * xlane_xl  where
     the lane coefficients remap the fetched span (ysel..+1) x (xsel..+1)
     onto the true bilinear corners including border clip/zero semantics.
  4. GPSIMD dma_gather (transpose=True) over a host-packed bf16 table
     x2[p] = [x[p], x[p+1], x[p+64], x[p+65]] ([P, 512] in DRAM): each
     int16 index fetches 1KB = all four bilinear corners x 128 channels,
     landing transposed as four [c, p] planes.  One gather per k.
  5. PE (bf16): per (k, corner, p-block): Z^T[p,o] = G[c,p-blk].T @ W_k[c,o]
     (gathered block as the stationary operand) -> PSUM [128, 4x128].
  6. DVE accumulates acc[p, o] += coef_corner[p] * Z^T straight from PSUM
     via scalar_tensor_tensor (per-partition scalar = per-position coef).
  7. Output [P, 128] (p-major) DMAd out; host transposes to [Cout, H, W].
"""

import sys

sys.path.insert(0, "/opt/trn_rl_repo")

import numpy as np
import ml_dtypes

import concourse.bacc as bacc
import concourse.bass as bass
import concourse.mybir as mybir
import concourse.tile as tile
from concourse.ap import AP
from concourse.bass import ts
from concourse.bass_utils import run_bass_kernel_spmd
from concourse.library_config import mlp as mlp_lib
from concourse.masks import make_identity

F32 = mybir.dt.float32
BF16 = mybir.dt.bfloat16
I16 = mybir.dt.int16

B, C, H, W = 8, 128, 64, 64
COUT = 128
K2 = 9
P = H * W            # 4096
NPB = P // 128       # 32 p-blocks
HP = H + 2           # padded side
MAGIC = 12582912.0   # 1.5 * 2**23
AOP = mybir.AluOpType
AF = mybir.ActivationFunctionType

_CACHE = {}


def _build():
    nc = bacc.Bacc("TRN2", target_bir_lowering=False, num_swdge_queues=4)

    xpad_d = nc.dram_tensor("xpad", [128, HP * HP], F32, kind="ExternalInput")
    x2_d = nc.dram_tensor("x2rows", [P, 512], BF16, kind="ExternalInput")
    wmain_d = nc.dram_tensor("wmain", [K2, 128, COUT], BF16, kind="ExternalInput")
    woff_d = nc.dram_tensor("woff", [K2, 128, 41], F32, kind="ExternalInput")
    bias_d = nc.dram_tensor("bias41", [41, 1], F32, kind="ExternalInput")
    byk_d = nc.dram_tensor("byk", [128, K2, NPB], F32, kind="ExternalInput")
    bxk_d = nc.dram_tensor("bxk", [128, K2, NPB], F32, kind="ExternalInput")
    out_d = nc.dram_tensor("out", [P, COUT], F32, kind="ExternalOutput")

    with tile.TileContext(nc) as tc:
        with (
            tc.tile_pool(name="const", bufs=1) as cp,
            tc.tile_pool(name="coef", bufs=1) as cf,
            tc.tile_pool(name="gp", bufs=2) as gp,
        ):
            from contextlib import ExitStack
            _es0 = ExitStack()
            nc.gpsimd.load_library(mlp_lib)

            # ---- constant loads (SP-engine HWDGE queues, off gpsimd) ----
            xpad = cp.tile([128, HP * HP], F32)
            nc.sync.dma_start(xpad[:, 0 : 10 * HP], xpad_d[:, 0 : 10 * HP])
            nc.sync.dma_start(xpad[:, 10 * HP :], xpad_d[:, 10 * HP :])
            wm = cp.tile([128, K2, COUT], BF16)
            nc.sync.dma_start(wm[:], wmain_d[:].rearrange("k c o -> c k o"))
            wo = cp.tile([128, K2, 41], F32)
            nc.sync.dma_start(wo[:], woff_d[:].rearrange("k c j -> c k j"))
            bias = cp.tile([41, 1], F32)
            nc.sync.dma_start(bias[:], bias_d[:])
            byk = cp.tile([128, K2, NPB], F32)
            nc.sync.dma_start(byk[:], byk_d[:])
            bxk = cp.tile([128, K2, NPB], F32)
            nc.sync.dma_start(bxk[:], bxk_d[:])
            ident = cp.tile([64, 64], F32)
            make_identity(nc, ident[:])
            # PE warm-up: ~4us of dummy matmuls so the HAM un-throttles
            # before the offset conv begins.
            wup = _es0.enter_context(tc.tile_pool(name="wup", bufs=1, space="PSUM"))
            wps = wup.tile([64, 512], F32)
            for _ in range(8):
                nc.tensor.matmul(
                    wps[:, 0:64], ident[:], ident[:], start=True, stop=True
                )

            # ---- offset/mask conv: [41, P] channel-major ----
            _es1 = ExitStack()
            psO_ctx = _es1.enter_context(tc.tile_pool(name="psO", bufs=2, space="PSUM"))
            psT_ctx = _es1.enter_context(tc.tile_pool(name="psT", bufs=2, space="PSUM"))
            offs_cm = cf.tile([41, P], F32)
            nc.gpsimd.memset(offs_cm[:], 0.0)
            xv = xpad[:].rearrange("c (h w) -> c h w", h=HP)
            for ch in range(8):
                po = psO_ctx.tile([41, 512], F32)
                r0 = ch * 8
                for k in range(K2):
                    ki, kj = k // 3, k % 3
                    rhs = xv[:, r0 + ki : r0 + ki + 8, kj : kj + W]
                    nc.tensor.matmul(
                        po[:], wo[:, k, :], rhs,
                        start=(k == 0), stop=(k == K2 - 1),
                    )
                sl = slice(ch * 512, (ch + 1) * 512)
                nc.scalar.activation(
                    offs_cm[0:18, sl], po[0:18, :], AF.Identity,
                    bias=bias[0:18, :], scale=1.0,
                )
                nc.scalar.activation(
                    offs_cm[32:41, sl], po[32:41, :], AF.Sigmoid,
                    bias=bias[32:41, :], scale=1.0,
                )

            # ---- transpose to p-major [128, 41, 32] ----
            offs_pm = cf.tile([128, 41, NPB], F32)
            for t in range(NPB):
                pt = psT_ctx.tile([128, 41], F32)
                nc.tensor.transpose(
                    pt[:], offs_cm[:, ts(t, 128)], ident[:41, :41]
                )
                nc.vector.tensor_copy(offs_pm[:, :, t], pt[:])

            offy = offs_pm[:, 0:9, :]
            offx = offs_pm[:, 9:18, :]
            mask = offs_pm[:, 32:41, :]

            # ---- coefficient planes (DVE, [128, 9, 32] each) ----
            SH = [128, K2, NPB]
            _tln = [0]

            def tl():
                _tln[0] += 1
                return cf.tile(SH, F32, name=f"cftmp{_tln[0]}")

            def TS(out, in0, s1, op0, s2=None, op1=None):
                kw = {"op1": op1} if op1 is not None else {}
                nc.vector.tensor_scalar(
                    out=out, in0=in0, scalar1=s1, scalar2=s2, op0=op0, **kw
                )

            def TT(out, a, b, op):
                nc.vector.tensor_tensor(out=out, in0=a, in1=b, op=op)

            # index chain first (gathers depend only on this)
            t0 = tl(); TS(t0[:], offy, -0.5, AOP.add, MAGIC, AOP.add)
            iy = tl(); TS(iy[:], t0[:], MAGIC, AOP.subtract)
            ys0 = tl(); TT(ys0[:], iy[:], byk[:], AOP.add)
            ysel = tl(); TS(ysel[:], ys0[:], 0.0, AOP.max, 62.0, AOP.min)
            t1 = tl(); TS(t1[:], offx, -0.5, AOP.add, MAGIC, AOP.add)
            ix = tl(); TS(ix[:], t1[:], MAGIC, AOP.subtract)
            xs0 = tl(); TT(xs0[:], ix[:], bxk[:], AOP.add)
            xst = tl(); TS(xst[:], xs0[:], 0.0, AOP.max, 62.0, AOP.min)
            ib = tl()
            nc.vector.scalar_tensor_tensor(
                out=ib[:], in0=ysel[:], scalar=64.0, in1=xst[:],
                op0=AOP.mult, op1=AOP.add,
            )
            idx16 = cf.tile([128, K2, NPB], I16)
            nc.vector.tensor_copy(idx16[:], ib[:])
            # wrap per-k across 3 HWDGE queues so gathers start asap
            idxw = cf.tile([128, K2, 256], I16)
            _dmaengs = (nc.sync, nc.sync)
            for k in range(K2):
                for g in range(8):
                    _dmaengs[g % 2].dma_start(
                        idxw[0:16, k, g:256:8],
                        idx16[16 * g : 16 * (g + 1), k, :],
                    )
                for qi, np2 in enumerate((16, 32, 64)):
                    _dmaengs[qi % 2].dma_start(
                        idxw[np2 : 2 * np2, k, :], idxw[0:np2, k, :]
                    )
            # remaining coefficient math
            fy = tl(); TT(fy[:], offy, iy[:], AOP.subtract)
            ys1 = tl(); TS(ys1[:], ys0[:], 1.0, AOP.add)
            yc0 = tl(); TS(yc0[:], ys0[:], 0.0, AOP.max, 63.0, AOP.min)
            yc1 = tl(); TS(yc1[:], ys1[:], 0.0, AOP.max, 63.0, AOP.min)
            vy0 = tl(); TT(vy0[:], yc0[:], ys0[:], AOP.is_equal)
            vy1 = tl(); TT(vy1[:], yc1[:], ys1[:], AOP.is_equal)
            gy = tl(); TS(gy[:], fy[:], -1.0, AOP.mult, 1.0, AOP.add)
            wy0 = tl(); TT(wy0[:], gy[:], vy0[:], AOP.mult)
            wy1 = tl(); TT(wy1[:], fy[:], vy1[:], AOP.mult)
            f0 = tl(); TT(f0[:], ysel[:], ys0[:], AOP.is_equal)
            fm = tl(); TS(fm[:], ys0[:], -1.0, AOP.is_equal)
            fp = tl(); TS(fp[:], ys0[:], 63.0, AOP.is_equal)
            ya = tl(); TT(ya[:], wy0[:], f0[:], AOP.mult)
            yb = tl(); TT(yb[:], wy1[:], fm[:], AOP.mult)
            ylane0 = tl(); TT(ylane0[:], ya[:], yb[:], AOP.add)
            yc_ = tl(); TT(yc_[:], wy1[:], f0[:], AOP.mult)
            yd = tl(); TT(yd[:], wy0[:], fp[:], AOP.mult)
            ylane1 = tl(); TT(ylane1[:], yc_[:], yd[:], AOP.add)
            myl0 = tl(); TT(myl0[:], ylane0[:], mask, AOP.mult)
            myl1 = tl(); TT(myl1[:], ylane1[:], mask, AOP.mult)
            # x side
            fx = tl(); TT(fx[:], offx, ix[:], AOP.subtract)
            xs1 = tl(); TS(xs1[:], xs0[:], 1.0, AOP.add)
            xc0 = tl(); TS(xc0[:], xs0[:], 0.0, AOP.max, 63.0, AOP.min)
            xc1 = tl(); TS(xc1[:], xs1[:], 0.0, AOP.max, 63.0, AOP.min)
            vx0 = tl(); TT(vx0[:], xc0[:], xs0[:], AOP.is_equal)
            vx1 = tl(); TT(vx1[:], xc1[:], xs1[:], AOP.is_equal)
            gx = tl(); TS(gx[:], fx[:], -1.0, AOP.mult, 1.0, AOP.add)
            wx0 = tl(); TT(wx0[:], gx[:], vx0[:], AOP.mult)
            wx1 = tl(); TT(wx1[:], fx[:], vx1[:], AOP.mult)
            e0 = tl(); TT(e0[:], xst[:], xs0[:], AOP.is_equal)
            em = tl(); TS(em[:], xs0[:], -1.0, AOP.is_equal)
            ep = tl(); TS(ep[:], xs0[:], 63.0, AOP.is_equal)
            l0a = tl(); TT(l0a[:], wx0[:], e0[:], AOP.mult)
            l0b = tl(); TT(l0b[:], wx1[:], em[:], AOP.mult)
            xlane0 = tl(); TT(xlane0[:], l0a[:], l0b[:], AOP.add)
            l1a = tl(); TT(l1a[:], wx1[:], e0[:], AOP.mult)
            l1b = tl(); TT(l1b[:], wx0[:], ep[:], AOP.mult)
            xlane1 = tl(); TT(xlane1[:], l1a[:], l1b[:], AOP.add)
            # final per-corner coefficients (gather quarter order:
            # (y0,x0), (y0,x1), (y1,x0), (y1,x1))
            C00 = tl(); TT(C00[:], myl0[:], xlane0[:], AOP.mult)
            C01 = tl(); TT(C01[:], myl0[:], xlane1[:], AOP.mult)
            C10 = tl(); TT(C10[:], myl1[:], xlane0[:], AOP.mult)
            C11 = tl(); TT(C11[:], myl1[:], xlane1[:], AOP.mult)

            _es1.close()
            _es0.close()
            _es2 = ExitStack()
            psZ = _es2.enter_context(tc.tile_pool(name="psZ", bufs=6, space="PSUM"))
            z2p = _es2.enter_context(tc.tile_pool(name="z2p", bufs=4))

            # ---- main loop ----
            acc = cf.tile([128, NPB, COUT], F32)
            nc.gpsimd.memset(acc[:], 0.0)

            src_ap = AP(
                tensor=x2_d[:].tensor, offset=0, ap=[[512, P], [1, 512]]
            )
            CPLANES = (C00, C01, C10, C11)
            NIDX_CHUNK = 512
            NCH = P // NIDX_CHUNK
            for k in range(K2):
                gt = gp.tile([128, NCH, 4, NIDX_CHUNK], BF16, tag="G")
                for c8 in range(NCH):
                    nc.gpsimd.dma_gather(
                        gt[:, c8, :, :],
                        src_ap,
                        idxw[:, k, c8 * 32 : (c8 + 1) * 32],
                        NIDX_CHUNK, NIDX_CHUNK,
                        elem_size=512, elem_step=512, transpose=True,
                        queue_num=0,
                    )
                for pb in range(NPB):
                    pz = psZ.tile([128, 512], F32)
                    c8, sub = pb // 4, pb % 4
                    for j in range(4):
                        nc.tensor.matmul(
                            pz[:, ts(j, 128)],
                            gt[:, c8, j, ts(sub, 128)],
                            wm[:, k, :],
                            start=True, stop=True,
                        )
                    # corners 0,1: ACT scaled-copies into PSUM (fast path);
                    # DVE: two fused STTs + two TTs, each with a single
                    # PSUM operand (walrus constraint)
                    z2 = z2p.tile([128, 2, 128], F32, tag="z2")
                    for j in range(2):
                        nc.scalar.activation(
                            z2[:, j, :], pz[:, ts(j, 128)], AF.Copy,
                            scale=CPLANES[j][:, k, pb : pb + 1],
                        )
                    u = z2p.tile([128, 2, 128], F32, tag="u")
                    for j in (2, 3):
                        nc.vector.scalar_tensor_tensor(
                            out=u[:, j - 2, :],
                            in0=pz[:, ts(j, 128)],
                            scalar=CPLANES[j][:, k, pb : pb + 1],
                            in1=z2[:, j - 2, :],
                            op0=AOP.mult, op1=AOP.add,
                        )
                    zt = z2p.tile([128, 128], F32, tag="zt")
                    nc.vector.tensor_tensor(
                        out=zt[:], in0=u[:, 0, :], in1=u[:, 1, :], op=AOP.add
                    )
                    nc.vector.tensor_tensor(
                        out=acc[:, pb, :], in0=zt[:], in1=acc[:, pb, :],
                        op=AOP.add,
                    )

            nc.sync.dma_start(
                out_d[:].rearrange("(pb part) o -> part pb o", part=128), acc[:]
            )
            _es2.close()

    nc.compile()
    return nc


def _host_prep(x, weight, offset_w, offset_b, mask_w, mask_b):
    x = np.asarray(x, np.float32)
    weight = np.asarray(weight, np.float32)
    offset_w = np.asarray(offset_w, np.float32)
    offset_b = np.asarray(offset_b, np.float32)
    mask_w = np.asarray(mask_w, np.float32)
    mask_b = np.asarray(mask_b, np.float32)

    wmain = np.ascontiguousarray(
        np.transpose(weight.reshape(COUT, C, K2), (2, 1, 0))
    ).astype(ml_dtypes.bfloat16)
    ow = offset_w.reshape(18, C, K2)
    w41 = np.zeros((41, C, K2), np.float32)
    w41[0:9] = ow[0::2]
    w41[9:18] = ow[1::2]
    w41[32:41] = mask_w.reshape(9, C, K2)
    woff = np.ascontiguousarray(np.transpose(w41, (2, 1, 0)))
    bias41 = np.zeros((41, 1), np.float32)
    bias41[0:9, 0] = offset_b[0::2]
    bias41[9:18, 0] = offset_b[1::2]
    bias41[32:41, 0] = mask_b

    ps = np.arange(P)
    ho = (ps // W).reshape(NPB, 128).T.astype(np.float32)
    wo_ = (ps % W).reshape(NPB, 128).T.astype(np.float32)
    byk = np.empty((128, K2, NPB), np.float32)
    bxk = np.empty((128, K2, NPB), np.float32)
    for k in range(K2):
        byk[:, k, :] = ho + (k // 3 - 1)
        bxk[:, k, :] = wo_ + (k % 3 - 1)

    shared = dict(wmain=wmain, woff=woff, bias41=bias41, byk=byk, bxk=bxk)

    in_maps = []
    for b in range(B):
        xpad = np.zeros((C, HP, HP), np.float32)
        xpad[:, 1 : H + 1, 1 : W + 1] = x[b]
        xr = np.zeros((P + 66, C), ml_dtypes.bfloat16)
        xr[:P] = x[b].transpose(1, 2, 0).reshape(P, C).astype(ml_dtypes.bfloat16)
        x2 = np.ascontiguousarray(
            np.concatenate(
                [xr[0:P], xr[1 : P + 1], xr[64 : P + 64], xr[65 : P + 65]],
                axis=1,
            )
        )
        in_maps.append(
            dict(xpad=xpad.reshape(C, HP * HP), x2rows=x2, **shared)
        )
    return in_maps


def kernel(x, weight, offset_w, offset_b, mask_w, mask_b):
    if "nc" not in _CACHE:
        _CACHE["nc"] = _build()
    nc = _CACHE["nc"]
    in_maps = _host_prep(x, weight, offset_w, offset_b, mask_w, mask_b)
    res = run_bass_kernel_spmd(nc, in_maps, list(range(B)))
    _CACHE["last_result"] = res
    out = np.empty((B, COUT, H, W), np.float32)
    for b in range(B):
        out[b] = res.results[b]["out"].T.reshape(COUT, H, W)
    return out


# revision 35
# speedup vs baseline: 1.2437x; 1.1832x over previous
"""DCNv2 (modulated deformable conv 3x3) for Trainium2, 8 NeuronCores.

Sharding: pure data-parallel over batch B=8 -> core b computes batch b.

Per-core algorithm (batch b, C=Cout=128, H=W=64, P=H*W=4096):
  1. PE (fp32): offset/mask conv as 9 accumulated matmuls over a zero-padded
     channel-major x ([128, 66*66] SBUF), output [41, P] channel-major
     (channels: 0:9 y-offsets, 9:18 x-offsets, 32:41 mask - 32-aligned for
     the engines' base-partition restriction).  ACT applies bias (+ sigmoid
     for mask rows) during PSUM evacuation.
  2. PE transposes [41,128] chunks -> p-major planes [128(p), 41, 32(pb)].
  3. DVE: bilinear coefficient planes.  floor() via the fp32 round trick
     (x - 0.5 + 1.5*2^23) - 1.5*2^23 (ties resolve either way; bilinear
     interpolation is continuous so both splits give identical samples).
     Per kernel-point k one gather index  idx = ysel*64 + xsel  with
     ysel = clip(floor(py), 0, 62), xsel = clip(floor(px), 0, 62), and four
     per-corner coefficients  C[yl][xl] = mask * ylane_yl * xlane_xl  where
     the lane coefficients remap the fetched span (ysel..+1) x (xsel..+1)
     onto the true bilinear corners including border clip/zero semantics.
  4. GPSIMD dma_gather (transpose=True) over a host-packed bf16 table
     x2[p] = [x[p], x[p+1], x[p+64], x[p+65]] ([P, 512] in DRAM): each
     int16 index fetches 1KB = all four bilinear corners x 128 channels,
     landing transposed as four [c, p] planes.  One gather per k.
  5. PE (bf16): per (k, corner, p-block): Z^T[p,o] = G[c,p-blk].T @ W_k[c,o]
     (gathered block as the stationary operand) -> PSUM [128, 4x128].
  6. DVE accumulates acc[p, o] += coef_corner[p] * Z^T straight from PSUM
     via scalar_tensor_tensor (per-partition scalar = per-position coef).
  7. Output [P, 128] (p-major) DMAd out; host transposes to [Cout, H, W].
"""

import sys

sys.path.insert(0, "/opt/trn_rl_repo")

import numpy as np
import ml_dtypes

import concourse.bacc as bacc
import concourse.bass as bass
import concourse.mybir as mybir
import concourse.tile as tile
from concourse.ap import AP
from concourse.bass import ts
from concourse.bass_utils import run_bass_kernel_spmd
from concourse.library_config import mlp as mlp_lib
from concourse.masks import make_identity

F32 = mybir.dt.float32
BF16 = mybir.dt.bfloat16
I16 = mybir.dt.int16

B, C, H, W = 8, 128, 64, 64
COUT = 128
K2 = 9
P = H * W            # 4096
NPB = P // 128       # 32 p-blocks
HP = H + 2           # padded side
MAGIC = 12582912.0   # 1.5 * 2**23
AOP = mybir.AluOpType
AF = mybir.ActivationFunctionType

_CACHE = {}


def _build():
    nc = bacc.Bacc("TRN2", target_bir_lowering=False, num_swdge_queues=4)

    xpad_d = nc.dram_tensor("xpad", [128, HP * HP], F32, kind="ExternalInput")
    x2_d = nc.dram_tensor("x2rows", [P, 512], BF16, kind="ExternalInput")
    wmain_d = nc.dram_tensor("wmain", [K2, 128, COUT], BF16, kind="ExternalInput")
    woff_d = nc.dram_tensor("woff", [K2, 128, 41], F32, kind="ExternalInput")
    bias_d = nc.dram_tensor("bias41", [41, 1], F32, kind="ExternalInput")
    byk_d = nc.dram_tensor("byk", [128, K2, NPB], F32, kind="ExternalInput")
    bxk_d = nc.dram_tensor("bxk", [128, K2, NPB], F32, kind="ExternalInput")
    out_d = nc.dram_tensor("out", [P, COUT], F32, kind="ExternalOutput")

    with tile.TileContext(nc) as tc:
        with (
            tc.tile_pool(name="const", bufs=1) as cp,
            tc.tile_pool(name="coef", bufs=1) as cf,
            tc.tile_pool(name="gp", bufs=2) as gp,
        ):
            from contextlib import ExitStack
            _es0 = ExitStack()
            nc.gpsimd.load_library(mlp_lib)

            # ---- constant loads (SP-engine HWDGE queues, off gpsimd) ----
            xpad = cp.tile([128, HP * HP], F32)
            nc.sync.dma_start(xpad[:, 0 : 10 * HP], xpad_d[:, 0 : 10 * HP])
            nc.sync.dma_start(xpad[:, 10 * HP :], xpad_d[:, 10 * HP :])
            wm = cp.tile([128, K2, COUT], BF16)
            nc.sync.dma_start(wm[:], wmain_d[:].rearrange("k c o -> c k o"))
            wo = cp.tile([128, K2, 41], F32)
            nc.sync.dma_start(wo[:], woff_d[:].rearrange("k c j -> c k j"))
            bias = cp.tile([41, 1], F32)
            nc.sync.dma_start(bias[:], bias_d[:])
            byk = cp.tile([128, K2, NPB], F32)
            nc.sync.dma_start(byk[:], byk_d[:])
            bxk = cp.tile([128, K2, NPB], F32)
            nc.sync.dma_start(bxk[:], bxk_d[:])
            ident = cp.tile([64, 64], F32)
            make_identity(nc, ident[:])
            # PE warm-up: ~4us of dummy matmuls so the HAM un-throttles
            # before the offset conv begins.
            wup = _es0.enter_context(tc.tile_pool(name="wup", bufs=1, space="PSUM"))
            wps = wup.tile([64, 512], F32)
            for _ in range(8):
                nc.tensor.matmul(
                    wps[:, 0:64], ident[:], ident[:], start=True, stop=True
                )

            # ---- offset/mask conv: [41, P] channel-major ----
            _es1 = ExitStack()
            psO_ctx = _es1.enter_context(tc.tile_pool(name="psO", bufs=2, space="PSUM"))
            psT_ctx = _es1.enter_context(tc.tile_pool(name="psT", bufs=2, space="PSUM"))
            offs_cm = cf.tile([41, P], F32)
            nc.gpsimd.memset(offs_cm[:], 0.0)
            xv = xpad[:].rearrange("c (h w) -> c h w", h=HP)
            for ch in range(8):
                po = psO_ctx.tile([41, 512], F32)
                r0 = ch * 8
                for k in range(K2):
                    ki, kj = k // 3, k % 3
                    rhs = xv[:, r0 + ki : r0 + ki + 8, kj : kj + W]
                    nc.tensor.matmul(
                        po[:], wo[:, k, :], rhs,
                        start=(k == 0), stop=(k == K2 - 1),
                    )
                sl = slice(ch * 512, (ch + 1) * 512)
                nc.scalar.activation(
                    offs_cm[0:18, sl], po[0:18, :], AF.Identity,
                    bias=bias[0:18, :], scale=1.0,
                )
                nc.scalar.activation(
                    offs_cm[32:41, sl], po[32:41, :], AF.Sigmoid,
                    bias=bias[32:41, :], scale=1.0,
                )

            # ---- transpose to p-major [128, 41, 32] ----
            offs_pm = cf.tile([128, 41, NPB], F32)
            for t in range(NPB):
                pt = psT_ctx.tile([128, 41], F32)
                nc.tensor.transpose(
                    pt[:], offs_cm[:, ts(t, 128)], ident[:41, :41]
                )
                nc.vector.tensor_copy(offs_pm[:, :, t], pt[:])

            offy = offs_pm[:, 0:9, :]
            offx = offs_pm[:, 9:18, :]
            mask = offs_pm[:, 32:41, :]

            # ---- coefficient planes (DVE, [128, 9, 32] each) ----
            SH = [128, K2, NPB]
            _tln = [0]

            def tl():
                _tln[0] += 1
                return cf.tile(SH, F32, name=f"cftmp{_tln[0]}")

            def TS(out, in0, s1, op0, s2=None, op1=None):
                kw = {"op1": op1} if op1 is not None else {}
                nc.vector.tensor_scalar(
                    out=out, in0=in0, scalar1=s1, scalar2=s2, op0=op0, **kw
                )

            def TT(out, a, b, op):
                nc.vector.tensor_tensor(out=out, in0=a, in1=b, op=op)

            # index chain first (gathers depend only on this)
            t0 = tl(); TS(t0[:], offy, -0.5, AOP.add, MAGIC, AOP.add)
            iy = tl(); TS(iy[:], t0[:], MAGIC, AOP.subtract)
            ys0 = tl(); TT(ys0[:], iy[:], byk[:], AOP.add)
            ysel = tl(); TS(ysel[:], ys0[:], 0.0, AOP.max, 62.0, AOP.min)
            t1 = tl(); TS(t1[:], offx, -0.5, AOP.add, MAGIC, AOP.add)
            ix = tl(); TS(ix[:], t1[:], MAGIC, AOP.subtract)
            xs0 = tl(); TT(xs0[:], ix[:], bxk[:], AOP.add)
            xst = tl(); TS(xst[:], xs0[:], 0.0, AOP.max, 62.0, AOP.min)
            ib = tl()
            nc.vector.scalar_tensor_tensor(
                out=ib[:], in0=ysel[:], scalar=64.0, in1=xst[:],
                op0=AOP.mult, op1=AOP.add,
            )
            idx16 = cf.tile([128, K2, NPB], I16)
            nc.vector.tensor_copy(idx16[:], ib[:])
            # wrap per-k across 3 HWDGE queues so gathers start asap
            idxw = cf.tile([128, K2, 256], I16)
            _dmaengs = (nc.sync, nc.sync)
            for k in range(K2):
                for g in range(8):
                    _dmaengs[g % 2].dma_start(
                        idxw[0:16, k, g:256:8],
                        idx16[16 * g : 16 * (g + 1), k, :],
                    )
                for qi, np2 in enumerate((16, 32, 64)):
                    _dmaengs[qi % 2].dma_start(
                        idxw[np2 : 2 * np2, k, :], idxw[0:np2, k, :]
                    )
            # remaining coefficient math
            fy = tl(); TT(fy[:], offy, iy[:], AOP.subtract)
            ys1 = tl(); TS(ys1[:], ys0[:], 1.0, AOP.add)
            yc0 = tl(); TS(yc0[:], ys0[:], 0.0, AOP.max, 63.0, AOP.min)
            yc1 = tl(); TS(yc1[:], ys1[:], 0.0, AOP.max, 63.0, AOP.min)
            vy0 = tl(); TT(vy0[:], yc0[:], ys0[:], AOP.is_equal)
            vy1 = tl(); TT(vy1[:], yc1[:], ys1[:], AOP.is_equal)
            gy = tl(); TS(gy[:], fy[:], -1.0, AOP.mult, 1.0, AOP.add)
            wy0 = tl(); TT(wy0[:], gy[:], vy0[:], AOP.mult)
            wy1 = tl(); TT(wy1[:], fy[:], vy1[:], AOP.mult)
            f0 = tl(); TT(f0[:], ysel[:], ys0[:], AOP.is_equal)
            fm = tl(); TS(fm[:], ys0[:], -1.0, AOP.is_equal)
            fp = tl(); TS(fp[:], ys0[:], 63.0, AOP.is_equal)
            ya = tl(); TT(ya[:], wy0[:], f0[:], AOP.mult)
            yb = tl(); TT(yb[:], wy1[:], fm[:], AOP.mult)
            ylane0 = tl(); TT(ylane0[:], ya[:], yb[:], AOP.add)
            yc_ = tl(); TT(yc_[:], wy1[:], f0[:], AOP.mult)
            yd = tl(); TT(yd[:], wy0[:], fp[:], AOP.mult)
            ylane1 = tl(); TT(ylane1[:], yc_[:], yd[:], AOP.add)
            myl0 = tl(); TT(myl0[:], ylane0[:], mask, AOP.mult)
            myl1 = tl(); TT(myl1[:], ylane1[:], mask, AOP.mult)
            # x side
            fx = tl(); TT(fx[:], offx, ix[:], AOP.subtract)
            xs1 = tl(); TS(xs1[:], xs0[:], 1.0, AOP.add)
            xc0 = tl(); TS(xc0[:], xs0[:], 0.0, AOP.max, 63.0, AOP.min)
            xc1 = tl(); TS(xc1[:], xs1[:], 0.0, AOP.max, 63.0, AOP.min)
            vx0 = tl(); TT(vx0[:], xc0[:], xs0[:], AOP.is_equal)
            vx1 = tl(); TT(vx1[:], xc1[:], xs1[:], AOP.is_equal)
            gx = tl(); TS(gx[:], fx[:], -1.0, AOP.mult, 1.0, AOP.add)
            wx0 = tl(); TT(wx0[:], gx[:], vx0[:], AOP.mult)
            wx1 = tl(); TT(wx1[:], fx[:], vx1[:], AOP.mult)
            e0 = tl(); TT(e0[:], xst[:], xs0[:], AOP.is_equal)
            em = tl(); TS(em[:], xs0[:], -1.0, AOP.is_equal)
            ep = tl(); TS(ep[:], xs0[:], 63.0, AOP.is_equal)
            l0a = tl(); TT(l0a[:], wx0[:], e0[:], AOP.mult)
            l0b = tl(); TT(l0b[:], wx1[:], em[:], AOP.mult)
            xlane0 = tl(); TT(xlane0[:], l0a[:], l0b[:], AOP.add)
            l1a = tl(); TT(l1a[:], wx1[:], e0[:], AOP.mult)
            l1b = tl(); TT(l1b[:], wx0[:], ep[:], AOP.mult)
            xlane1 = tl(); TT(xlane1[:], l1a[:], l1b[:], AOP.add)
            # final per-corner coefficients (gather quarter order:
            # (y0,x0), (y0,x1), (y1,x0), (y1,x1))
            C00 = tl(); TT(C00[:], myl0[:], xlane0[:], AOP.mult)
            C01 = tl(); TT(C01[:], myl0[:], xlane1[:], AOP.mult)
            C10 = tl(); TT(C10[:], myl1[:], xlane0[:], AOP.mult)
            C11 = tl(); TT(C11[:], myl1[:], xlane1[:], AOP.mult)

            _es1.close()
            _es0.close()
            _es2 = ExitStack()
            psZ = _es2.enter_context(tc.tile_pool(name="psZ", bufs=6, space="PSUM"))
            z2p = _es2.enter_context(tc.tile_pool(name="z2p", bufs=4))

            # ---- main loop ----
            acc = cf.tile([128, NPB, COUT], F32)
            nc.gpsimd.memset(acc[:], 0.0)

            src_ap = AP(
                tensor=x2_d[:].tensor, offset=0, ap=[[512, P], [1, 512]]
            )
            CPLANES = (C00, C01, C10, C11)
            NIDX_CHUNK = 512
            NCH = P // NIDX_CHUNK
            for k in range(K2):
                gt = gp.tile([128, NCH, 4, NIDX_CHUNK], BF16, tag="G")
                for c8 in range(NCH):
                    nc.gpsimd.dma_gather(
                        gt[:, c8, :, :],
                        src_ap,
                        idxw[:, k, c8 * 32 : (c8 + 1) * 32],
                        NIDX_CHUNK, NIDX_CHUNK,
                        elem_size=512, elem_step=512, transpose=True,
                        queue_num=(k * NCH + c8) % 4,
                    )
                for pb in range(NPB):
                    pz = psZ.tile([128, 512], F32)
                    c8, sub = pb // 4, pb % 4
                    for j in range(4):
                        nc.tensor.matmul(
                            pz[:, ts(j, 128)],
                            gt[:, c8, j, ts(sub, 128)],
                            wm[:, k, :],
                            start=True, stop=True,
                        )
                    # corners 0,1: ACT scaled-copies into PSUM (fast path);
                    # DVE: two fused STTs + two TTs, each with a single
                    # PSUM operand (walrus constraint)
                    z2 = z2p.tile([128, 2, 128], F32, tag="z2")
                    for j in range(2):
                        nc.scalar.activation(
                            z2[:, j, :], pz[:, ts(j, 128)], AF.Copy,
                            scale=CPLANES[j][:, k, pb : pb + 1],
                        )
                    u = z2p.tile([128, 2, 128], F32, tag="u")
                    for j in (2, 3):
                        nc.vector.scalar_tensor_tensor(
                            out=u[:, j - 2, :],
                            in0=pz[:, ts(j, 128)],
                            scalar=CPLANES[j][:, k, pb : pb + 1],
                            in1=z2[:, j - 2, :],
                            op0=AOP.mult, op1=AOP.add,
                        )
                    zt = z2p.tile([128, 128], F32, tag="zt")
                    nc.vector.tensor_tensor(
                        out=zt[:], in0=u[:, 0, :], in1=u[:, 1, :], op=AOP.add
                    )
                    nc.vector.tensor_tensor(
                        out=acc[:, pb, :], in0=zt[:], in1=acc[:, pb, :],
                        op=AOP.add,
                    )

            nc.sync.dma_start(
                out_d[:].rearrange("(pb part) o -> part pb o", part=128), acc[:]
            )
            _es2.close()

    nc.compile()
    return nc


def _host_prep(x, weight, offset_w, offset_b, mask_w, mask_b):
    x = np.asarray(x, np.float32)
    weight = np.asarray(weight, np.float32)
    offset_w = np.asarray(offset_w, np.float32)
    offset_b = np.asarray(offset_b, np.float32)
    mask_w = np.asarray(mask_w, np.float32)
    mask_b = np.asarray(mask_b, np.float32)

    wmain = np.ascontiguousarray(
        np.transpose(weight.reshape(COUT, C, K2), (2, 1, 0))
    ).astype(ml_dtypes.bfloat16)
    ow = offset_w.reshape(18, C, K2)
    w41 = np.zeros((41, C, K2), np.float32)
    w41[0:9] = ow[0::2]
    w41[9:18] = ow[1::2]
    w41[32:41] = mask_w.reshape(9, C, K2)
    woff = np.ascontiguousarray(np.transpose(w41, (2, 1, 0)))
    bias41 = np.zeros((41, 1), np.float32)
    bias41[0:9, 0] = offset_b[0::2]
    bias41[9:18, 0] = offset_b[1::2]
    bias41[32:41, 0] = mask_b

    ps = np.arange(P)
    ho = (ps // W).reshape(NPB, 128).T.astype(np.float32)
    wo_ = (ps % W).reshape(NPB, 128).T.astype(np.float32)
    byk = np.empty((128, K2, NPB), np.float32)
    bxk = np.empty((128, K2, NPB), np.float32)
    for k in range(K2):
        byk[:, k, :] = ho + (k // 3 - 1)
        bxk[:, k, :] = wo_ + (k % 3 - 1)

    shared = dict(wmain=wmain, woff=woff, bias41=bias41, byk=byk, bxk=bxk)

    in_maps = []
    for b in range(B):
        xpad = np.zeros((C, HP, HP), np.float32)
        xpad[:, 1 : H + 1, 1 : W + 1] = x[b]
        xr = np.zeros((P + 66, C), ml_dtypes.bfloat16)
        xr[:P] = x[b].transpose(1, 2, 0).reshape(P, C).astype(ml_dtypes.bfloat16)
        x2 = np.ascontiguousarray(
            np.concatenate(
                [xr[0:P], xr[1 : P + 1], xr[64 : P + 64], xr[65 : P + 65]],
                axis=1,
            )
        )
        in_maps.append(
            dict(xpad=xpad.reshape(C, HP * HP), x2rows=x2, **shared)
        )
    return in_maps


def kernel(x, weight, offset_w, offset_b, mask_w, mask_b):
    if "nc" not in _CACHE:
        _CACHE["nc"] = _build()
    nc = _CACHE["nc"]
    in_maps = _host_prep(x, weight, offset_w, offset_b, mask_w, mask_b)
    res = run_bass_kernel_spmd(nc, in_maps, list(range(B)))
    _CACHE["last_result"] = res
    out = np.empty((B, COUT, H, W), np.float32)
    for b in range(B):
        out[b] = res.results[b]["out"].T.reshape(COUT, H, W)
    return out


# revision 36
# speedup vs baseline: 1.2885x; 1.0361x over previous
"""DCNv2 (modulated deformable conv 3x3) for Trainium2, 8 NeuronCores.

Sharding: pure data-parallel over batch B=8 -> core b computes batch b.

Per-core algorithm (batch b, C=Cout=128, H=W=64, P=H*W=4096):
  1. PE (fp32): offset/mask conv as 9 accumulated matmuls over a zero-padded
     channel-major x ([128, 66*66] SBUF), output [41, P] channel-major
     (channels: 0:9 y-offsets, 9:18 x-offsets, 32:41 mask - 32-aligned for
     the engines' base-partition restriction).  ACT applies bias (+ sigmoid
     for mask rows) during PSUM evacuation.
  2. PE transposes [41,128] chunks -> p-major planes [128(p), 41, 32(pb)].
  3. DVE: bilinear coefficient planes.  floor() via the fp32 round trick
     (x - 0.5 + 1.5*2^23) - 1.5*2^23 (ties resolve either way; bilinear
     interpolation is continuous so both splits give identical samples).
     Per kernel-point k one gather index  idx = ysel*64 + xsel  with
     ysel = clip(floor(py), 0, 62), xsel = clip(floor(px), 0, 62), and four
     per-corner coefficients  C[yl][xl] = mask * ylane_yl * xlane_xl  where
     the lane coefficients remap the fetched span (ysel..+1) x (xsel..+1)
     onto the true bilinear corners including border clip/zero semantics.
  4. GPSIMD dma_gather (transpose=True) over a host-packed bf16 table
     x2[p] = [x[p], x[p+1], x[p+64], x[p+65]] ([P, 512] in DRAM): each
     int16 index fetches 1KB = all four bilinear corners x 128 channels,
     landing transposed as four [c, p] planes.  One gather per k.
  5. PE (bf16): per (k, corner, p-block): Z^T[p,o] = G[c,p-blk].T @ W_k[c,o]
     (gathered block as the stationary operand) -> PSUM [128, 4x128].
  6. DVE accumulates acc[p, o] += coef_corner[p] * Z^T straight from PSUM
     via scalar_tensor_tensor (per-partition scalar = per-position coef).
  7. Output [P, 128] (p-major) DMAd out; host transposes to [Cout, H, W].
"""

import sys

sys.path.insert(0, "/opt/trn_rl_repo")

import numpy as np
import ml_dtypes

import concourse.bacc as bacc
import concourse.bass as bass
import concourse.mybir as mybir
import concourse.tile as tile
from concourse.ap import AP
from concourse.bass import ts
from concourse.bass_utils import run_bass_kernel_spmd
from concourse.library_config import mlp as mlp_lib
from concourse.masks import make_identity

F32 = mybir.dt.float32
BF16 = mybir.dt.bfloat16
I16 = mybir.dt.int16

B, C, H, W = 8, 128, 64, 64
COUT = 128
K2 = 9
P = H * W            # 4096
NPB = P // 128       # 32 p-blocks
HP = H + 2           # padded side
MAGIC = 12582912.0   # 1.5 * 2**23
AOP = mybir.AluOpType
AF = mybir.ActivationFunctionType

_CACHE = {}


def _build():
    nc = bacc.Bacc("TRN2", target_bir_lowering=False, num_swdge_queues=4)

    xpad_d = nc.dram_tensor("xpad", [128, HP * HP], F32, kind="ExternalInput")
    x2_d = nc.dram_tensor("x2rows", [P, 512], BF16, kind="ExternalInput")
    wmain_d = nc.dram_tensor("wmain", [K2, 128, COUT], BF16, kind="ExternalInput")
    woff_d = nc.dram_tensor("woff", [K2, 128, 41], F32, kind="ExternalInput")
    bias_d = nc.dram_tensor("bias41", [41, 1], F32, kind="ExternalInput")
    byk_d = nc.dram_tensor("byk", [128, K2, NPB], F32, kind="ExternalInput")
    bxk_d = nc.dram_tensor("bxk", [128, K2, NPB], F32, kind="ExternalInput")
    out_d = nc.dram_tensor("out", [P, COUT], F32, kind="ExternalOutput")

    with tile.TileContext(nc) as tc:
        with (
            tc.tile_pool(name="const", bufs=1) as cp,
            tc.tile_pool(name="coef", bufs=1) as cf,
            tc.tile_pool(name="gp", bufs=2) as gp,
        ):
            from contextlib import ExitStack
            _es0 = ExitStack()
            nc.gpsimd.load_library(mlp_lib)

            # ---- constant loads (SP-engine HWDGE queues, off gpsimd) ----
            xpad = cp.tile([128, HP * HP], F32)
            wo = cp.tile([128, K2, 41], F32)
            nc.sync.dma_start(wo[:], woff_d[:].rearrange("k c j -> c k j"))
            nc.sync.dma_start(xpad[:, 0 : 10 * HP], xpad_d[:, 0 : 10 * HP])
            nc.sync.dma_start(xpad[:, 10 * HP :], xpad_d[:, 10 * HP :])
            wm = cp.tile([128, K2, COUT], BF16)
            nc.sync.dma_start(wm[:], wmain_d[:].rearrange("k c o -> c k o"))
            bias = cp.tile([41, 1], F32)
            nc.sync.dma_start(bias[:], bias_d[:])
            byk = cp.tile([128, K2, NPB], F32)
            nc.sync.dma_start(byk[:], byk_d[:])
            bxk = cp.tile([128, K2, NPB], F32)
            nc.sync.dma_start(bxk[:], bxk_d[:])
            ident = cp.tile([64, 64], F32)
            make_identity(nc, ident[:])

            # ---- offset/mask conv: [41, P] channel-major ----
            _es1 = ExitStack()
            psO_ctx = _es1.enter_context(tc.tile_pool(name="psO", bufs=2, space="PSUM"))
            psT_ctx = _es1.enter_context(tc.tile_pool(name="psT", bufs=2, space="PSUM"))
            offs_cm = cf.tile([41, P], F32)
            nc.gpsimd.memset(offs_cm[:], 0.0)
            xv = xpad[:].rearrange("c (h w) -> c h w", h=HP)
            for ch in range(8):
                po = psO_ctx.tile([41, 512], F32)
                r0 = ch * 8
                for k in range(K2):
                    ki, kj = k // 3, k % 3
                    rhs = xv[:, r0 + ki : r0 + ki + 8, kj : kj + W]
                    nc.tensor.matmul(
                        po[:], wo[:, k, :], rhs,
                        start=(k == 0), stop=(k == K2 - 1),
                    )
                sl = slice(ch * 512, (ch + 1) * 512)
                nc.scalar.activation(
                    offs_cm[0:18, sl], po[0:18, :], AF.Identity,
                    bias=bias[0:18, :], scale=1.0,
                )
                nc.scalar.activation(
                    offs_cm[32:41, sl], po[32:41, :], AF.Sigmoid,
                    bias=bias[32:41, :], scale=1.0,
                )

            # ---- transpose to p-major [128, 41, 32] ----
            offs_pm = cf.tile([128, 41, NPB], F32)
            for t in range(NPB):
                pt = psT_ctx.tile([128, 41], F32)
                nc.tensor.transpose(
                    pt[:], offs_cm[:, ts(t, 128)], ident[:41, :41]
                )
                nc.vector.tensor_copy(offs_pm[:, :, t], pt[:])

            offy = offs_pm[:, 0:9, :]
            offx = offs_pm[:, 9:18, :]
            mask = offs_pm[:, 32:41, :]

            # ---- coefficient planes (DVE, [128, 9, 32] each) ----
            SH = [128, K2, NPB]
            _tln = [0]

            def tl():
                _tln[0] += 1
                return cf.tile(SH, F32, name=f"cftmp{_tln[0]}")

            def TS(out, in0, s1, op0, s2=None, op1=None):
                kw = {"op1": op1} if op1 is not None else {}
                nc.vector.tensor_scalar(
                    out=out, in0=in0, scalar1=s1, scalar2=s2, op0=op0, **kw
                )

            def TT(out, a, b, op):
                nc.vector.tensor_tensor(out=out, in0=a, in1=b, op=op)

            # index chain first (gathers depend only on this)
            t0 = tl(); TS(t0[:], offy, -0.5, AOP.add, MAGIC, AOP.add)
            iy = tl(); TS(iy[:], t0[:], MAGIC, AOP.subtract)
            ys0 = tl(); TT(ys0[:], iy[:], byk[:], AOP.add)
            ysel = tl(); TS(ysel[:], ys0[:], 0.0, AOP.max, 62.0, AOP.min)
            t1 = tl(); TS(t1[:], offx, -0.5, AOP.add, MAGIC, AOP.add)
            ix = tl(); TS(ix[:], t1[:], MAGIC, AOP.subtract)
            xs0 = tl(); TT(xs0[:], ix[:], bxk[:], AOP.add)
            xst = tl(); TS(xst[:], xs0[:], 0.0, AOP.max, 62.0, AOP.min)
            ib = tl()
            nc.vector.scalar_tensor_tensor(
                out=ib[:], in0=ysel[:], scalar=64.0, in1=xst[:],
                op0=AOP.mult, op1=AOP.add,
            )
            idx16 = cf.tile([128, K2, NPB], I16)
            nc.vector.tensor_copy(idx16[:], ib[:])
            # wrap per-k across 3 HWDGE queues so gathers start asap
            idxw = cf.tile([128, K2, 256], I16)
            _dmaengs = (nc.sync, nc.sync)
            for k in range(K2):
                for g in range(8):
                    _dmaengs[g % 2].dma_start(
                        idxw[0:16, k, g:256:8],
                        idx16[16 * g : 16 * (g + 1), k, :],
                    )
                for qi, np2 in enumerate((16, 32, 64)):
                    _dmaengs[qi % 2].dma_start(
                        idxw[np2 : 2 * np2, k, :], idxw[0:np2, k, :]
                    )
            # remaining coefficient math
            fy = tl(); TT(fy[:], offy, iy[:], AOP.subtract)
            ys1 = tl(); TS(ys1[:], ys0[:], 1.0, AOP.add)
            yc0 = tl(); TS(yc0[:], ys0[:], 0.0, AOP.max, 63.0, AOP.min)
            yc1 = tl(); TS(yc1[:], ys1[:], 0.0, AOP.max, 63.0, AOP.min)
            vy0 = tl(); TT(vy0[:], yc0[:], ys0[:], AOP.is_equal)
            vy1 = tl(); TT(vy1[:], yc1[:], ys1[:], AOP.is_equal)
            gy = tl(); TS(gy[:], fy[:], -1.0, AOP.mult, 1.0, AOP.add)
            wy0 = tl(); TT(wy0[:], gy[:], vy0[:], AOP.mult)
            wy1 = tl(); TT(wy1[:], fy[:], vy1[:], AOP.mult)
            f0 = tl(); TT(f0[:], ysel[:], ys0[:], AOP.is_equal)
            fm = tl(); TS(fm[:], ys0[:], -1.0, AOP.is_equal)
            fp = tl(); TS(fp[:], ys0[:], 63.0, AOP.is_equal)
            ya = tl(); TT(ya[:], wy0[:], f0[:], AOP.mult)
            yb = tl(); TT(yb[:], wy1[:], fm[:], AOP.mult)
            ylane0 = tl(); TT(ylane0[:], ya[:], yb[:], AOP.add)
            yc_ = tl(); TT(yc_[:], wy1[:], f0[:], AOP.mult)
            yd = tl(); TT(yd[:], wy0[:], fp[:], AOP.mult)
            ylane1 = tl(); TT(ylane1[:], yc_[:], yd[:], AOP.add)
            myl0 = tl(); TT(myl0[:], ylane0[:], mask, AOP.mult)
            myl1 = tl(); TT(myl1[:], ylane1[:], mask, AOP.mult)
            # x side
            fx = tl(); TT(fx[:], offx, ix[:], AOP.subtract)
            xs1 = tl(); TS(xs1[:], xs0[:], 1.0, AOP.add)
            xc0 = tl(); TS(xc0[:], xs0[:], 0.0, AOP.max, 63.0, AOP.min)
            xc1 = tl(); TS(xc1[:], xs1[:], 0.0, AOP.max, 63.0, AOP.min)
            vx0 = tl(); TT(vx0[:], xc0[:], xs0[:], AOP.is_equal)
            vx1 = tl(); TT(vx1[:], xc1[:], xs1[:], AOP.is_equal)
            gx = tl(); TS(gx[:], fx[:], -1.0, AOP.mult, 1.0, AOP.add)
            wx0 = tl(); TT(wx0[:], gx[:], vx0[:], AOP.mult)
            wx1 = tl(); TT(wx1[:], fx[:], vx1[:], AOP.mult)
            e0 = tl(); TT(e0[:], xst[:], xs0[:], AOP.is_equal)
            em = tl(); TS(em[:], xs0[:], -1.0, AOP.is_equal)
            ep = tl(); TS(ep[:], xs0[:], 63.0, AOP.is_equal)
            l0a = tl(); TT(l0a[:], wx0[:], e0[:], AOP.mult)
            l0b = tl(); TT(l0b[:], wx1[:], em[:], AOP.mult)
            xlane0 = tl(); TT(xlane0[:], l0a[:], l0b[:], AOP.add)
            l1a = tl(); TT(l1a[:], wx1[:], e0[:], AOP.mult)
            l1b = tl(); TT(l1b[:], wx0[:], ep[:], AOP.mult)
            xlane1 = tl(); TT(xlane1[:], l1a[:], l1b[:], AOP.add)
            # final per-corner coefficients (gather quarter order:
            # (y0,x0), (y0,x1), (y1,x0), (y1,x1))
            C00 = tl(); TT(C00[:], myl0[:], xlane0[:], AOP.mult)
            C01 = tl(); TT(C01[:], myl0[:], xlane1[:], AOP.mult)
            C10 = tl(); TT(C10[:], myl1[:], xlane0[:], AOP.mult)
            C11 = tl(); TT(C11[:], myl1[:], xlane1[:], AOP.mult)

            _es1.close()
            _es0.close()
            _es2 = ExitStack()
            psZ = _es2.enter_context(tc.tile_pool(name="psZ", bufs=6, space="PSUM"))
            z2p = _es2.enter_context(tc.tile_pool(name="z2p", bufs=4))

            # ---- main loop ----
            acc = cf.tile([128, NPB, COUT], F32)
            nc.gpsimd.memset(acc[:], 0.0)

            src_ap = AP(
                tensor=x2_d[:].tensor, offset=0, ap=[[512, P], [1, 512]]
            )
            CPLANES = (C00, C01, C10, C11)
            NIDX_CHUNK = 512
            NCH = P // NIDX_CHUNK
            for k in range(K2):
                gt = gp.tile([128, NCH, 4, NIDX_CHUNK], BF16, tag="G")
                for c8 in range(NCH):
                    nc.gpsimd.dma_gather(
                        gt[:, c8, :, :],
                        src_ap,
                        idxw[:, k, c8 * 32 : (c8 + 1) * 32],
                        NIDX_CHUNK, NIDX_CHUNK,
                        elem_size=512, elem_step=512, transpose=True,
                        queue_num=(k * NCH + c8) % 4,
                    )
                for pb in range(NPB):
                    pz = psZ.tile([128, 512], F32)
                    c8, sub = pb // 4, pb % 4
                    for j in range(4):
                        nc.tensor.matmul(
                            pz[:, ts(j, 128)],
                            gt[:, c8, j, ts(sub, 128)],
                            wm[:, k, :],
                            start=True, stop=True,
                        )
                    # corners 0,1: ACT scaled-copies into PSUM (fast path);
                    # DVE: two fused STTs + two TTs, each with a single
                    # PSUM operand (walrus constraint)
                    z2 = z2p.tile([128, 2, 128], F32, tag="z2")
                    for j in range(2):
                        nc.scalar.activation(
                            z2[:, j, :], pz[:, ts(j, 128)], AF.Copy,
                            scale=CPLANES[j][:, k, pb : pb + 1],
                        )
                    u = z2p.tile([128, 2, 128], F32, tag="u")
                    for j in (2, 3):
                        nc.vector.scalar_tensor_tensor(
                            out=u[:, j - 2, :],
                            in0=pz[:, ts(j, 128)],
                            scalar=CPLANES[j][:, k, pb : pb + 1],
                            in1=z2[:, j - 2, :],
                            op0=AOP.mult, op1=AOP.add,
                        )
                    zt = z2p.tile([128, 128], F32, tag="zt")
                    nc.vector.tensor_tensor(
                        out=zt[:], in0=u[:, 0, :], in1=u[:, 1, :], op=AOP.add
                    )
                    nc.vector.tensor_tensor(
                        out=acc[:, pb, :], in0=zt[:], in1=acc[:, pb, :],
                        op=AOP.add,
                    )

            nc.sync.dma_start(
                out_d[:].rearrange("(pb part) o -> part pb o", part=128), acc[:]
            )
            _es2.close()

    nc.compile()
    return nc


def _host_prep(x, weight, offset_w, offset_b, mask_w, mask_b):
    x = np.asarray(x, np.float32)
    weight = np.asarray(weight, np.float32)
    offset_w = np.asarray(offset_w, np.float32)
    offset_b = np.asarray(offset_b, np.float32)
    mask_w = np.asarray(mask_w, np.float32)
    mask_b = np.asarray(mask_b, np.float32)

    wmain = np.ascontiguousarray(
        np.transpose(weight.reshape(COUT, C, K2), (2, 1, 0))
    ).astype(ml_dtypes.bfloat16)
    ow = offset_w.reshape(18, C, K2)
    w41 = np.zeros((41, C, K2), np.float32)
    w41[0:9] = ow[0::2]
    w41[9:18] = ow[1::2]
    w41[32:41] = mask_w.reshape(9, C, K2)
    woff = np.ascontiguousarray(np.transpose(w41, (2, 1, 0)))
    bias41 = np.zeros((41, 1), np.float32)
    bias41[0:9, 0] = offset_b[0::2]
    bias41[9:18, 0] = offset_b[1::2]
    bias41[32:41, 0] = mask_b

    ps = np.arange(P)
    ho = (ps // W).reshape(NPB, 128).T.astype(np.float32)
    wo_ = (ps % W).reshape(NPB, 128).T.astype(np.float32)
    byk = np.empty((128, K2, NPB), np.float32)
    bxk = np.empty((128, K2, NPB), np.float32)
    for k in range(K2):
        byk[:, k, :] = ho + (k // 3 - 1)
        bxk[:, k, :] = wo_ + (k % 3 - 1)

    shared = dict(wmain=wmain, woff=woff, bias41=bias41, byk=byk, bxk=bxk)

    in_maps = []
    for b in range(B):
        xpad = np.zeros((C, HP, HP), np.float32)
        xpad[:, 1 : H + 1, 1 : W + 1] = x[b]
        xr = np.zeros((P + 66, C), ml_dtypes.bfloat16)
        xr[:P] = x[b].transpose(1, 2, 0).reshape(P, C).astype(ml_dtypes.bfloat16)
        x2 = np.ascontiguousarray(
            np.concatenate(
                [xr[0:P], xr[1 : P + 1], xr[64 : P + 64], xr[65 : P + 65]],
                axis=1,
            )
        )
        in_maps.append(
            dict(xpad=xpad.reshape(C, HP * HP), x2rows=x2, **shared)
        )
    return in_maps


def kernel(x, weight, offset_w, offset_b, mask_w, mask_b):
    if "nc" not in _CACHE:
        _CACHE["nc"] = _build()
    nc = _CACHE["nc"]
    in_maps = _host_prep(x, weight, offset_w, offset_b, mask_w, mask_b)
    res = run_bass_kernel_spmd(nc, in_maps, list(range(B)))
    _CACHE["last_result"] = res
    out = np.empty((B, COUT, H, W), np.float32)
    for b in range(B):
        out[b] = res.results[b]["out"].T.reshape(COUT, H, W)
    return out
